# revision 15
# baseline (speedup 1.0000x reference)
"""Banded multi-head attention (B=2, L=1024, D=1024, H=16, band W=64) on 8
Trainium2 NeuronCores.

Sharding: core = (batch b, head-group g) with 2 batches x 4 head groups of 4
heads each.  Each core computes q/k/v projections for its group, the banded
attention for its 4 heads, and a partial output projection through its slice
of Wo.  Host sums the 4 partial outputs per batch.

Schedule (v2, bf16):
- All matmul operands are bf16 (psum accumulation stays fp32): halves HBM
  traffic and keeps the PE at 1 cycle/row.  Verified numerically: bf16
  operands give ~5e-3 relative error vs the 2e-2 gate.
- Band attention runs on 128-key chunks: for key chunk c (keys 128(c-1)..
  128c-1 in padded coordinates), the in-band queries span at most 191
  columns, so scores for a head pair live in one [128, 2, <=191] psum tile.
  The band mask in (key-row r, query-col j) coordinates is position
  independent (in-band iff 0 <= j - r <= 63), so a single [128, 2, 191]
  mask tile serves every chunk including the cache chunk c=0, whose zero
  padded rows fall outside the band automatically.
- Per (head, T-half) softmax denominators ride along as a ones-column in V;
  1/d comes from vector.reciprocal, is partition-broadcast on GPSIMD, and
  scales the attention output on DVE.  No Ln/Exp round trip, no broadcast
  matmul on the PE.
- Engine balance: Act does exp + q/k psum->sbuf copies (+ some y copies),
  DVE does half the mask stt, reciprocal and the normalizing multiplies,
  GPSIMD (otherwise idle) does the other half of the stt, V copies,
  partition broadcasts and most y copies.
- DMA count is minimized (shared HWDGE is ~0.6us per transfer): Wq/Wk/Wv
  ship as one fused [D, 3*DG] tensor (8 chunk DMAs), x as 16 half-row
  chunks, y as 8 [128, 1024] bf16 stores.
"""
import numpy as np
import ml_dtypes

import concourse.bacc as bacc
import concourse.mybir as mybir
import concourse.tile as tile
from concourse import bass_utils

B, L, D, H, W = 2, 1024, 1024, 16, 64
DH = D // H           # 64
G = 4                 # head groups
HPG = H // G          # 4 heads per group
DG = D // G           # 256 dims per group
NCORES = 8

F32 = mybir.dt.float32
BF16 = mybir.dt.bfloat16
NEG = -1.0e30
EXPF = mybir.ActivationFunctionType.Exp
SCALE = float(DH) ** -0.5

NCHUNK = 9            # key chunks: c=0 cache block, 1..8 token chunks
SPAN = 191            # max in-band query span per key chunk
KTW = 128 + L         # kT padded width per head
VSLOT = DH + 1        # 65 cols per (slot, head): 64 v dims + ones
VROW = HPG * VSLOT    # 260 cols per slot


def _pin_exp_table(arch: str):
    """Resolve Copy/Exp/Identity only to the natural_log_exp_and_others
    act-func set so exactly one table load is emitted (alternating per-
    function table swaps wedge the device)."""
    import concourse.hw_specs as hw_specs
    tables = hw_specs.get_activation_tables(arch)   # cached, mutable
    drop = {EXPF, mybir.ActivationFunctionType.Copy,
            mybir.ActivationFunctionType.Identity}
    assert "natural_log_exp_and_others" in tables
    for name, funcs in tables.items():
        if name != "natural_log_exp_and_others":
            funcs -= drop


def _chunk_qspan(c):
    """Query range [qlo, qhi) covered by key chunk c, plus the mask column
    offset jlo (j = q - 128*(c-1))."""
    qlo = max(0, 128 * c - 128)
    qhi = min(L, 128 * c + 63)
    return qlo, qhi, qlo - (128 * c - 128)


def build(repeat: int = 1, loop_n: int = 0, staggered: bool = True):
    nc = bacc.Bacc("TRN2", target_bir_lowering=False, debug=False)
    _pin_exp_table(nc.m.arch)

    xT = nc.dram_tensor("xT", [D, L], BF16, kind="ExternalInput")
    wqkv = nc.dram_tensor("wqkv", [D, 3 * DG], BF16, kind="ExternalInput")
    woT = nc.dram_tensor("woT", [DG, D], BF16, kind="ExternalInput")
    kc = nc.dram_tensor("kc", [DH, HPG * 128], BF16, kind="ExternalInput")
    vc = nc.dram_tensor("vc", [128, VROW], BF16, kind="ExternalInput")
    onesr = nc.dram_tensor("onesr", [128, 32], BF16, kind="ExternalInput")
    maskd = nc.dram_tensor("mask", [128, 2 * SPAN], F32, kind="ExternalInput")
    y = nc.dram_tensor("y", [L, D], BF16, kind="ExternalOutput")

    with tile.TileContext(nc) as tc:
        with tc.tile_pool(name="res", bufs=1) as res, \
             tc.tile_pool(name="epool", bufs=20) as epool, \
             tc.tile_pool(name="rcpool", bufs=4) as rcpool, \
             tc.tile_pool(name="bcpool", bufs=4) as bcpool, \
             tc.tile_pool(name="ypool", bufs=3) as ypool, \
             tc.tile_pool(name="ps", bufs=8, space="PSUM") as psp:

            # ---- resident SBUF tensors ----------------------------------
            # x and wqkv chunk PAIRS share one tile so each DMA covers two
            # 128-row chunks (halves HWDGE transfer count).
            xcat = [res.tile([128, 2, L], BF16, tag=f"xc{k2}", name=f"xc{k2}")
                    for k2 in range(4)]
            xk = [xcat[k // 2][:, k % 2, :] for k in range(8)]
            wcat = [res.tile([128, 2, 3 * DG], BF16, tag=f"wc{k2}",
                             name=f"wc{k2}") for k2 in range(4)]
            wk3 = [wcat[k // 2][:, k % 2, :] for k in range(8)]
            wo_sb = [res.tile([128, D], BF16, tag=f"wo{m}", name=f"wo{m}")
                     for m in range(2)]
            qT = res.tile([64, HPG * L], BF16, tag="qT", name="qT")
            kT = res.tile([64, HPG * KTW], BF16, tag="kT", name="kT")
            v_sb = res.tile([128, NCHUNK * VROW], BF16, tag="v", name="v_sb")
            mask_sb = res.tile([128, 2, SPAN], F32, tag="mask", name="mask_sb")
            oT = [res.tile([128, L], BF16, tag=f"oT{m}", name=f"oT{m}")
                  for m in range(2)]

            def wslice(k, which):
                return wk3[k][:, which * DG:(which + 1) * DG]

            def emit_qk_group(which, dst, off, wid, m, n):
                """Project x through Wq/Wk chunk column block m for token
                half n; write per-head slices of dst ([64, HPG*wid])."""
                pt = psp.tile([128, 512], F32, tag="ps", name="pj")
                for k in range(8):
                    nc.tensor.matmul(
                        pt[:],
                        wslice(k, which)[:, m * 128:(m + 1) * 128],
                        xk[k][:, n * 512:(n + 1) * 512],
                        start=(k == 0), stop=(k == 7),
                    )
                for hh in range(2):
                    h = 2 * m + hh
                    dsl = dst[:, h * wid + off + n * 512:
                              h * wid + off + n * 512 + 512]
                    if hh == 0:
                        nc.scalar.copy(dsl, pt[0:64, :])
                    else:
                        nc.vector.tensor_copy(dsl, pt[64:128, :])

            def emit_v(t):
                pv = psp.tile([128, 512], F32, tag="ps", name="pjv")
                for k in range(8):
                    nc.tensor.matmul(
                        pv[:, 0:DG],
                        xk[k][:, t * 128:(t + 1) * 128],
                        wslice(k, 2),
                        start=(k == 0), stop=(k == 7),
                    )
                si = t + 1
                dst = v_sb[:, si * VROW:(si + 1) * VROW].rearrange(
                    "p (h c) -> p h c", c=VSLOT)[:, :, 0:DH]
                nc.scalar.copy(
                    dst, pv[:, 0:DG].rearrange("p (h c) -> p h c", c=DH))

            def emit_scores(m, c):
                """Scores for head pair m, key chunk c -> exp'd bf16 tile."""
                qlo, qhi, jlo = _chunk_qspan(c)
                span = qhi - qlo
                st = psp.tile([128, 2, SPAN], F32, tag="ps", name="st")
                for hh in range(2):
                    h = 2 * m + hh
                    nc.tensor.matmul(
                        st[:, hh, 0:span],
                        kT[:, h * KTW + 128 * c: h * KTW + 128 * c + 128],
                        qT[:, h * L + qlo: h * L + qhi],
                        start=True, stop=True,
                    )
                nc.vector.scalar_tensor_tensor(
                    st[:, :, 0:span], st[:, :, 0:span], SCALE,
                    mask_sb[:, :, jlo:jlo + span],
                    mybir.AluOpType.mult, mybir.AluOpType.add,
                )
                e = epool.tile([128, 2, SPAN], BF16, tag="e", name="e")
                nc.scalar.activation(e[:, :, 0:span], st[:, :, 0:span], EXPF)
                return e

            def emit_attnv(m, T, es):
                """Attention @ V for head pair m, query half T using the
                per-chunk exp tiles es[c]; returns psum tiles per head.

                PSUM pending-zero semantics: start=True marks the whole 2KB
                bank pending; any write to a pending byte zeroes it first.
                So per chunk emit the accumulate piece (into the range the
                PREVIOUS chunk's start just cleared) before this chunk's own
                start piece, which re-marks the bank."""
                base = 512 * T
                ops = []
                for hh in range(2):
                    hg = 2 * m + hh
                    op = psp.tile([65, 512], F32, tag="ps", name="op")
                    mms = []
                    prev_hi = base
                    for c in range(4 * T, 4 * T + 5):
                        qlo, qhi, _ = _chunk_qspan(c)
                        olo = max(qlo, base)
                        ohi = min(qhi, base + 512)
                        if olo < prev_hi:          # accumulate piece
                            mms.append((c, qlo, olo, prev_hi, False))
                        if ohi > prev_hi:          # fresh (start) piece
                            mms.append((c, qlo, prev_hi, ohi, True))
                        prev_hi = max(prev_hi, ohi)
                    for i, (c, qlo, lo, hi, st) in enumerate(mms):
                        nc.tensor.matmul(
                            op[:, lo - base: hi - base],
                            v_sb[:, c * VROW + hg * VSLOT:
                                 c * VROW + hg * VSLOT + VSLOT],
                            es[c][:, hh, lo - qlo: hi - qlo],
                            start=st, stop=(i == len(mms) - 1),
                            skip_group_check=True,
                        )
                    ops.append(op)
                return ops

            def emit_norm(m, T, ops):
                """Scale attention outputs by 1/denominator into oT."""
                for hh in range(2):
                    op = ops[hh]
                    rc = rcpool.tile([1, 512], F32, tag="rc", name="rc")
                    nc.vector.reciprocal(rc[:], op[64:65, :])
                    bc = bcpool.tile([64, 512], F32, tag="bc", name="bc")
                    nc.gpsimd.partition_broadcast(bc[:], rc[:])
                    nc.vector.tensor_mul(
                        oT[m][hh * 64:(hh + 1) * 64, T * 512:(T + 1) * 512],
                        op[0:64, :], bc[:])

            def emit_oproj(tp):
                """Output projection for token-tile pair (2*tp, 2*tp+1);
                one [256-row] DMA per pair halves the HWDGE transfer count."""
                ysb = ypool.tile([128, 2, 1024], BF16, tag="y", name="ysb")
                for ti in range(2):
                    t = 2 * tp + ti
                    for n2 in range(2):
                        yp = psp.tile([128, 512], F32, tag="ps", name="yp")
                        for m in range(2):
                            nc.tensor.matmul(
                                yp[:],
                                oT[m][:, t * 128:(t + 1) * 128],
                                wo_sb[m][:, n2 * 512:(n2 + 1) * 512],
                                start=(m == 0), stop=(m == 1),
                            )
                        dsl = ysb[:, ti, n2 * 512:(n2 + 1) * 512]
                        nc.scalar.copy(dsl, yp[:])
                nc.sync.dma_start(
                    y.ap()[2 * tp * 128:(2 * tp + 2) * 128, :].rearrange(
                        "(u p) n -> p u n", u=2),
                    ysb[:])

            import contextlib

            def rep_ctx():
                if loop_n:
                    return tc.For_i(0, loop_n, 1,
                                    staggered_reset=staggered,
                                    hint_engines=(mybir.EngineType.PE,
                                                  mybir.EngineType.Activation,
                                                  mybir.EngineType.DVE,
                                                  mybir.EngineType.Pool,
                                                  mybir.EngineType.SP))
                return contextlib.nullcontext()

            # ---- loop-invariant loads (once, before the repeat loop) -----
            # Mask, K/V caches and the V ones-columns are never overwritten
            # by compute (V copies only touch cols 0..63 of each slot), so
            # they load once even when the body runs under For_i.
            nc.gpsimd.dma_start(
                mask_sb[:], maskd.ap().rearrange("p (s n) -> p s n", s=2))
            nc.gpsimd.dma_start(
                kT[:].rearrange("p (h c) -> p h c", c=KTW)[:, :, 0:128],
                kc.ap().rearrange("p (h c) -> p h c", c=128),
            )
            nc.gpsimd.dma_start(v_sb[:, 0:VROW], vc.ap())
            ones_cols = v_sb[:, VROW:].rearrange(
                "p (n c) -> p n c", c=VSLOT)[:, :, DH:DH + 1]
            nc.gpsimd.dma_start(
                ones_cols, onesr.ap()[:, 0:32].unsqueeze(2))

            with rep_ctx():
              for rep in range(repeat):
                # ---- input DMAs (weights + x, K-chunk-pair interleaved) --
                for k2 in range(4):
                    nc.sync.dma_start(
                        wcat[k2][:],
                        wqkv.ap()[k2 * 256:(k2 + 1) * 256, :].rearrange(
                            "(u p) n -> p u n", u=2))
                    nc.sync.dma_start(
                        xcat[k2][:, :, 0:512],
                        xT.ap()[k2 * 256:(k2 + 1) * 256, 0:512].rearrange(
                            "(u p) n -> p u n", u=2))
                for m in range(2):
                    nc.sync.dma_start(wo_sb[m][:],
                                      woT.ap()[m * 128:(m + 1) * 128, :])
                for k2 in range(4):
                    nc.sync.dma_start(
                        xcat[k2][:, :, 512:1024],
                        xT.ap()[k2 * 256:(k2 + 1) * 256, 512:1024].rearrange(
                            "(u p) n -> p u n", u=2))

                # ---- projections + attention, overlapped -----------------
                # Pair-0 scores for chunks 0..3 only need token half 0 of
                # qT/kT, so they slot in between the n=0 and n=1 projection
                # groups and keep Act/DVE busy under the n=1 matmuls.
                for m in range(2):
                    emit_qk_group(0, qT, 0, L, m, 0)
                    emit_qk_group(1, kT, 128, KTW, m, 0)
                emit_v(0)
                emit_v(1)
                emit_v(2)
                emit_v(3)
                es0, es1 = {}, {}
                for c in range(4):
                    es0[c] = emit_scores(0, c)
                    es1[c] = emit_scores(1, c)
                for m in range(2):
                    emit_qk_group(0, qT, 0, L, m, 1)
                    emit_qk_group(1, kT, 128, KTW, m, 1)
                es0[4] = emit_scores(0, 4)
                ops0 = emit_attnv(0, 0, es0)
                emit_norm(0, 0, ops0)
                emit_v(4)
                emit_v(5)
                emit_v(6)
                emit_v(7)
                for c in range(5, NCHUNK):
                    es0[c] = emit_scores(0, c)
                ops1 = emit_attnv(0, 1, es0)
                emit_norm(0, 1, ops1)

                es1[4] = emit_scores(1, 4)
                ops0 = emit_attnv(1, 0, es1)
                emit_norm(1, 0, ops0)
                for c in range(5, NCHUNK):
                    es1[c] = emit_scores(1, c)
                emit_oproj(0)
                emit_oproj(1)
                ops1 = emit_attnv(1, 1, es1)
                emit_norm(1, 1, ops1)

                # ---- output projection (T0 pairs were emitted above) -----
                for tp in range(2, 4):
                    emit_oproj(tp)

    nc.compile()
    return nc


def make_mask() -> np.ndarray:
    """[128, 2*SPAN] additive mask in chunk coordinates: key row r, query
    col j (query index q = 128*(c-1) + j); in-band iff 0 <= j - r <= 63.
    Doubled for the two heads sharing one score tile."""
    r = np.arange(128)[:, None]
    j = np.arange(SPAN)[None, :]
    m = np.where((j - r >= 0) & (j - r <= 63), 0.0, NEG).astype(np.float32)
    return np.concatenate([m, m], axis=1)


def prep_inputs(x, Wq, Wk, Wv, Wo, last_k_init, last_v_init):
    """Shard + pre-transpose full inputs into 8 per-core input maps."""
    bf = ml_dtypes.bfloat16
    mask = make_mask()
    in_maps = []
    for core in range(NCORES):
        b, g = divmod(core, G)
        sl = slice(g * DG, (g + 1) * DG)
        lk = last_k_init[:, g * HPG:(g + 1) * HPG, :]   # [63, 4, 64]
        lv = last_v_init[:, g * HPG:(g + 1) * HPG, :]
        # cache K block per head: [64, 128] with cols 0..64 zero,
        # 65..127 = keys -63..-1; stacked [64, 4*128]
        kcg = np.zeros((DH, HPG * 128), dtype=np.float32)
        for h in range(HPG):
            kcg[:, h * 128 + 65: h * 128 + 128] = lk[:, h, :].T
        vcg = np.zeros((128, VROW), dtype=np.float32)
        for h in range(HPG):
            vcg[65:128, h * VSLOT:h * VSLOT + DH] = lv[:, h, :]
            vcg[65:128, h * VSLOT + DH] = 1.0
        wqkv = np.concatenate(
            [Wq[sl, :].T, Wk[sl, :].T, Wv[sl, :].T], axis=1)  # [D, 3*DG]
        in_maps.append({
            "xT": np.ascontiguousarray(x[b].T).astype(bf),
            "wqkv": np.ascontiguousarray(wqkv).astype(bf),
            "woT": np.ascontiguousarray(Wo[:, sl].T).astype(bf),
            "kc": kcg.astype(bf),
            "vc": vcg.astype(bf),
            "onesr": np.ones((128, 32), dtype=np.float32).astype(bf),
            "mask": mask,
        })
    return in_maps


_built = None


def kernel(x, Wq, Wk, Wv, Wo, last_k_init, last_v_init) -> np.ndarray:
    global _built
    x = np.asarray(x, dtype=np.float32)
    args = [np.asarray(a, dtype=np.float32)
            for a in (Wq, Wk, Wv, Wo, last_k_init, last_v_init)]
    in_maps = prep_inputs(x, *args)
    if _built is None:
        _built = build()
    r = bass_utils.run_bass_kernel_spmd(
        _built, in_maps, core_ids=list(range(NCORES)))
    out = np.zeros((B, L, D), dtype=np.float32)
    for core in range(NCORES):
        b = core // G
        out[b] += np.asarray(r.results[core]["y"], dtype=np.float32)
    return out
